# revision 51
# baseline (speedup 1.0000x reference)
import numpy as np

import concourse.bass as bass
from concourse import bacc
import concourse.mybir as mybir
import concourse.tile as tile
from concourse.bass_utils import run_bass_kernel_spmd

# ---- problem constants (hardcoded) ----
D = 256; NH = 8; NL = 4; NP = 4; DFF = 1024; BS = 8; NQ = 300
DH = D // NH  # 32
EPS = 1e-5
SPATIAL = np.array([[100, 150], [50, 75], [25, 38], [13, 19]], dtype=np.int64)
SIZES = (SPATIAL[:, 0] * SPATIAL[:, 1])
S = int(SIZES.sum())  # 19947
LSI = np.concatenate([[0], np.cumsum(SIZES)[:-1]]).astype(np.int64)
ST = S + 1  # table padded (pair windows read j, j+1); even

F32 = mybir.dt.float32
F32R = mybir.dt.float32r
BF16 = mybir.dt.bfloat16
U16 = mybir.dt.uint16
ALU = mybir.AluOpType
ACT = mybir.ActivationFunctionType
AX = mybir.AxisListType

QT = [(0, 128), (128, 128), (256, 44)]
GCH = [(i * 32, 32) for i in range(9)] + [(288, 16)]
WQPAD = 320
NSEL = 8
SUBW = 2560

_cache = {}
import os
SKIP = set(os.environ.get('KSKIP','').split(','))


USE_F32R = False


def _r(ap):
    return ap.bitcast(F32R) if USE_F32R else ap


def build_bass():
    nc = bacc.Bacc("TRN2", target_bir_lowering=False)
    tgtT = nc.dram_tensor("tgtT", [D, NQ], F32, kind="ExternalInput")
    posT = nc.dram_tensor("posT", [D, NQ], F32, kind="ExternalInput")
    memT = nc.dram_tensor("memT", [D, S], F32, kind="ExternalInput")
    refs = nc.dram_tensor("refs", [NQ, 8], F32, kind="ExternalInput")
    wqT = nc.dram_tensor("wqT", [D, D], F32, kind="ExternalInput")
    wkT = nc.dram_tensor("wkT", [D, D], F32, kind="ExternalInput")
    wvT = nc.dram_tensor("wvT", [D, D], F32, kind="ExternalInput")
    qb = nc.dram_tensor("qb", [D, 1], F32, kind="ExternalInput")
    kb = nc.dram_tensor("kb", [D, 1], F32, kind="ExternalInput")
    woT = nc.dram_tensor("woT", [D, D], F32, kind="ExternalInput")
    wob = nc.dram_tensor("wob", [D, 1], F32, kind="ExternalInput")
    wsoT = nc.dram_tensor("wsoT", [D, D], F32, kind="ExternalInput")
    wawT = nc.dram_tensor("wawT", [D, 128], F32, kind="ExternalInput")
    wvdT = nc.dram_tensor("wvdT", [D, D], F32, kind="ExternalInput")
    vdb = nc.dram_tensor("vdb", [D, 1], F32, kind="ExternalInput")
    wodT = nc.dram_tensor("wodT", [D, D], F32, kind="ExternalInput")
    wodb = nc.dram_tensor("wodb", [D, 1], F32, kind="ExternalInput")
    w1T = nc.dram_tensor("w1T", [D, DFF], F32, kind="ExternalInput")
    b1 = nc.dram_tensor("b1", [DFF, 1], F32, kind="ExternalInput")
    w2T = nc.dram_tensor("w2T", [DFF, D], F32, kind="ExternalInput")
    b2 = nc.dram_tensor("b2", [D, 1], F32, kind="ExternalInput")
    ln_gb = nc.dram_tensor("ln_gb", [D, 6], F32, kind="ExternalInput")
    consts = nc.dram_tensor("consts", [6, 128], F32, kind="ExternalInput")
    consts2 = nc.dram_tensor("consts2", [2, D], F32, kind="ExternalInput")
    ident_in = nc.dram_tensor("ident_in", [128, 128], F32, kind="ExternalInput")
    sel_in = nc.dram_tensor("sel_in", [NH * NSEL, 128 * NSEL], F32, kind="ExternalInput")
    outT = nc.dram_tensor("outT", [D, NQ], F32, kind="ExternalOutput")
    wdram = nc.dram_tensor("wdram", [NH, WQPAD * 64], F32)
    jdram = nc.dram_tensor("jdram", [2 * 4 * 2 * 16 * 304], U16)

    with tile.TileContext(nc) as tc:
        import contextlib
        ctx = contextlib.ExitStack()
        with ctx:
            single = ctx.enter_context(tc.tile_pool(name="single", bufs=1))
            actp = ctx.enter_context(tc.tile_pool(name="actp", bufs=1))
            tmp = ctx.enter_context(tc.tile_pool(name="tmp", bufs=2))
            mpool = ctx.enter_context(tc.tile_pool(name="mpool", bufs=3))
            gpool = ctx.enter_context(tc.tile_pool(name="gpool", bufs=2))
            ppool = ctx.enter_context(tc.tile_pool(name="ppool", bufs=2, space="PSUM"))
            vpool = ctx.enter_context(tc.tile_pool(name="vpool", bufs=2, space="PSUM"))
            

            # stacked loader: DRAM [K*128, X] -> SBUF [128, K, X]
            def loadS(dram, rows, cols, pool=single):
                k = rows // 128
                nm = "w_" + dram.name
                t = pool.tile([128, k, cols], F32, name=nm, tag=nm)
                for i in range(k):
                    nc.sync.dma_start(out=t[:, i, :], in_=dram[i * 128:(i + 1) * 128, :])
                return t

            def loadC(dram, rows, cols, pool=single):  # small col tiles [rows<=128, cols]
                nm = "w_" + dram.name
                t = pool.tile([rows, cols], F32, name=nm, tag=nm)
                nc.sync.dma_start(out=t[:, :], in_=dram[:, :])
                return t

            sb_wq = loadS(wqT, D, D); sb_wk = loadS(wkT, D, D); sb_wv = loadS(wvT, D, D)
            sb_qb = loadS(qb, D, 1); sb_kb = loadS(kb, D, 1)
            sb_wo = loadS(woT, D, D); sb_wob = loadS(wob, D, 1)
            sb_wso = loadS(wsoT, D, D); sb_waw = loadS(wawT, D, 128)
            sb_wvd = single.tile([128, 2, D], F32, name="w_wvdT", tag="w_wvdT")
            for i in range(2):
                nc.sync.dma_start(out=sb_wvd[:, i, :].bitcast(F32R), in_=wvdT[i * 128:(i + 1) * 128, :].bitcast(F32R))
            sb_vdb = loadS(vdb, D, 1)
            sb_wod = loadS(wodT, D, D); sb_wodb = loadS(wodb, D, 1)
            sb_b1 = loadS(b1, DFF, 1)
            sb_b2 = loadS(b2, D, 1)
            sb_lngb = loadS(ln_gb, D, 6)
            sb_ident = loadC(ident_in, 128, 128)
            sb_sel = loadC(sel_in, NH * NSEL, 128 * NSEL)

            sb_consts = single.tile([128, 6, 128], F32)
            cap = consts[:, :]
            nc.sync.dma_start(out=sb_consts[:, :, :],
                              in_=bass.AP(tensor=cap.tensor, offset=cap.offset, ap=[[0, 128], [128, 6], [1, 128]]))
            WLr = sb_consts[:, 0, :]; HLr = sb_consts[:, 1, :]
            WM2r = sb_consts[:, 2, :]; HM2r = sb_consts[:, 3, :]; LSIr = sb_consts[:, 4, :]
            AWBr = sb_consts[:, 5, :]
            sb_consts2 = single.tile([128, 2, D], F32)
            c2 = consts2[:, :]
            nc.sync.dma_start(out=sb_consts2[:, :, :],
                              in_=bass.AP(tensor=c2.tensor, offset=c2.offset, ap=[[0, 128], [D, 2], [1, D]]))
            SOBr = sb_consts2[:, 0, :]; VBSr = sb_consts2[:, 1, :]

            ones_col = single.tile([128, 1], F32)
            nc.vector.memset(ones_col[:, :], 1.0)
            ones_row = single.tile([1, 128], F32)
            nc.vector.memset(ones_row[:, :], 1.0)

            # activations stacked [128, 2, NQ]
            def newact():
                return actp.tile([128, 2, NQ], F32, tag="acts", name="acts", bufs=4)

            sb_tgt = actp.tile([128, 2, NQ], F32)
            sb_pos = actp.tile([128, 2, NQ], F32)
            for i in range(2):
                nc.sync.dma_start(out=sb_tgt[:, i, :], in_=tgtT[i * 128:(i + 1) * 128, :])
                nc.sync.dma_start(out=sb_pos[:, i, :], in_=posT[i * 128:(i + 1) * 128, :])

            def layer_norm(xT, gi, bi, out_t):
                ps_s = ppool.tile([1, NQ], F32, tag="ps", name="ps_s")
                ps_q = ppool.tile([1, NQ], F32, tag="ps", name="ps_q")
                for kk in range(2):
                    nc.tensor.matmul(ps_s[:, :], ones_col[:, :], xT[:, kk, :], start=(kk == 0), stop=(kk == 1))
                for kk in range(2):
                    sq = tmp.tile([128, NQ], F32, tag="lnsq", name="sq", bufs=1)
                    nc.vector.tensor_tensor(out=sq[:, :], in0=xT[:, kk, :], in1=xT[:, kk, :], op=ALU.mult)
                    nc.tensor.matmul(ps_q[:, :], ones_col[:, :], sq[:, :], start=(kk == 0), stop=(kk == 1))
                mean = tmp.tile([1, NQ], F32, tag="lnrow", bufs=4)
                nc.vector.tensor_scalar(out=mean[:, :], in0=ps_s[:, :], scalar1=1.0 / D, scalar2=None, op0=ALU.mult)
                var = tmp.tile([1, NQ], F32, tag="lnrow", bufs=4)
                nc.vector.tensor_scalar(out=var[:, :], in0=ps_q[:, :], scalar1=1.0 / D, scalar2=None, op0=ALU.mult)
                m2 = tmp.tile([1, NQ], F32, tag="lnrow", bufs=4)
                nc.vector.tensor_tensor(out=m2[:, :], in0=mean[:, :], in1=mean[:, :], op=ALU.mult)
                nc.vector.tensor_tensor(out=var[:, :], in0=var[:, :], in1=m2[:, :], op=ALU.subtract)
                nc.vector.tensor_scalar(out=var[:, :], in0=var[:, :], scalar1=EPS, scalar2=None, op0=ALU.add)
                nc.scalar.sqrt(out=var[:, :], in_=var[:, :])
                rstd = tmp.tile([1, NQ], F32, tag="lnrow", bufs=4)
                nc.vector.reciprocal(out=rstd[:, :], in_=var[:, :])
                nmr = tmp.tile([1, NQ], F32, tag="lnrow", bufs=4)
                nc.vector.tensor_tensor(out=nmr[:, :], in0=mean[:, :], in1=rstd[:, :], op=ALU.mult)
                nc.vector.tensor_scalar(out=nmr[:, :], in0=nmr[:, :], scalar1=-1.0, scalar2=None, op0=ALU.mult)
                ps_r = ppool.tile([128, NQ], F32, tag="ps", name="ps_r")
                ps_m = ppool.tile([128, NQ], F32, tag="ps", name="ps_m")
                nc.tensor.matmul(ps_r[:, :], ones_row[:, :], rstd[:, :], start=True, stop=True)
                nc.tensor.matmul(ps_m[:, :], ones_row[:, :], nmr[:, :], start=True, stop=True)
                for kk in range(2):
                    sl = slice(kk * 128, (kk + 1) * 128)
                    t1 = tmp.tile([128, NQ], F32, tag="lnt", name="t1", bufs=1)
                    nc.vector.tensor_tensor(out=t1[:, :], in0=xT[:, kk, :], in1=ps_r[:, :], op=ALU.mult)
                    nc.vector.tensor_tensor(out=t1[:, :], in0=t1[:, :], in1=ps_m[:, :], op=ALU.add)
                    nc.vector.tensor_scalar(out=out_t[:, kk, :], in0=t1[:, :],
                                            scalar1=sb_lngb[:, kk, gi:gi + 1], scalar2=sb_lngb[:, kk, bi:bi + 1],
                                            op0=ALU.mult, op1=ALU.add)
                return out_t

            # ================= self attention =================
            A = newact()
            for kk in range(2):
                nc.vector.tensor_tensor(out=A[:, kk, :], in0=sb_tgt[:, kk, :], in1=sb_pos[:, kk, :], op=ALU.add)

            def proj_T(src, w, bias_col, dst=None):
                if dst is None:
                    dst = newact()
                for mt in range(2):
                    ps = ppool.tile([128, NQ], F32, tag="ps")
                    for kk in range(2):
                        nc.tensor.matmul(ps[:, :], _r(w[:, kk, mt * 128:(mt + 1) * 128]),
                                         _r(src[:, kk, :]), start=(kk == 0), stop=(kk == 1))
                    if bias_col is not None:
                        nc.vector.tensor_scalar(out=dst[:, mt, :], in0=ps[:, :], scalar1=bias_col[:, mt, :],
                                                scalar2=None, op0=ALU.add)
                    else:
                        nc.vector.tensor_copy(out=dst[:, mt, :], in_=ps[:, :])
                return dst

            # packed q/k: 3 heads per tile at bases 0/32/64
            qk3 = {"q": [actp.tile([96, NQ], F32, name=f"q3_{i}") for i in range(3)],
                   "k": [actp.tile([96, NQ], F32, name=f"k3_{i}") for i in range(3)]}
            def qk_sl(which, h, cols):
                return qk3[which][h // 3][(h % 3) * DH:(h % 3) * DH + DH, cols]
            for w_, b_, which in ((sb_wq, sb_qb, "q"), (sb_wk, sb_kb, "k")):
                for mt in range(2):
                    ps = ppool.tile([128, NQ], F32, tag="ps", name="ps")
                    for kk in range(2):
                        nc.tensor.matmul(ps[:, :], _r(w_[:, kk, mt * 128:(mt + 1) * 128]),
                                         _r(A[:, kk, :]), start=(kk == 0), stop=(kk == 1))
                    for hh in range(4):
                        h = mt * 4 + hh
                        nc.vector.tensor_scalar(out=qk_sl(which, h, slice(None)),
                                                in0=ps[hh * DH:(hh + 1) * DH, :],
                                                scalar1=b_[hh * DH:(hh + 1) * DH, mt, :], scalar2=None, op0=ALU.add)
            v_nat = []
            for (q0, qn) in QT:
                ps = ppool.tile([128, D], F32, tag="ps")
                for kk in range(2):
                    nc.tensor.matmul(ps[:qn, :], _r(sb_tgt[:, kk, q0:q0 + qn]), _r(sb_wv[:, kk, :]),
                                     start=(kk == 0), stop=(kk == 1))
                vt = actp.tile([128, D], F32, tag="vnat", name="vt", bufs=3)
                nc.vector.tensor_tensor(out=vt[:qn, :], in0=ps[:qn, :], in1=VBSr[:qn, :], op=ALU.add)
                v_nat.append(vt)

            OT = newact()
            if 'attn' in SKIP:
                for kk in range(2):
                    nc.vector.memset(OT[:, kk, :], 0.0)
            for h in (range(NH) if 'attn' not in SKIP else []):
                kc = (h * DH) // 128
                ko = (h * DH) % 128
                attn = []
                for (q0, qn) in QT:
                    ps = ppool.tile([128, NQ], F32, tag="ps", name="ps")
                    nc.tensor.matmul(ps[:qn, :], _r(qk_sl("q", h, slice(q0, q0 + qn))), _r(qk_sl("k", h, slice(None))),
                                     start=True, stop=True)
                    mx = tmp.tile([128, 1], F32, tag="srow", bufs=3)
                    nc.vector.tensor_reduce(out=mx[:qn, :], in_=ps[:qn, :], op=ALU.max, axis=AX.X)
                    nc.vector.tensor_scalar(out=mx[:qn, :], in0=mx[:qn, :], scalar1=-1.0, scalar2=None, op0=ALU.mult)
                    ex = tmp.tile([128, NQ], F32, tag="sattn", name="ex", bufs=3)
                    nc.scalar.activation(out=ex[:qn, :], in_=ps[:qn, :], func=ACT.Exp, bias=mx[:qn, :])
                    sm = tmp.tile([128, 1], F32, tag="srow", bufs=3)
                    nc.vector.tensor_reduce(out=sm[:qn, :], in_=ex[:qn, :], op=ALU.add, axis=AX.X)
                    rc = tmp.tile([128, 1], F32, tag="srow", bufs=3)
                    nc.vector.reciprocal(out=rc[:qn, :], in_=sm[:qn, :])
                    nc.vector.tensor_scalar(out=ex[:qn, :], in0=ex[:qn, :], scalar1=rc[:qn, :], scalar2=None,
                                            op0=ALU.mult)
                    attn.append((ex, q0, qn))
                attnT = []
                for (k0, kn) in QT:
                    at = tmp.tile([128, NQ], F32, tag="sattnT", name="at", bufs=3)
                    for (aw_, q0, qn) in attn:
                        pst = ppool.tile([128, 128], F32, tag="pst", name="pst", bufs=1)
                        nc.tensor.transpose(pst[:kn, :qn], aw_[:qn, k0:k0 + kn], sb_ident[:qn, :qn])
                        nc.vector.tensor_copy(out=at[:kn, q0:q0 + qn], in_=pst[:kn, :qn])
                    attnT.append((at, k0, kn))
                ps_o = ppool.tile([DH, NQ], F32, tag="pso", name="ps_o", bufs=1)
                for ci, (at, k0, kn) in enumerate(attnT):
                    nc.tensor.matmul(ps_o[:, :], _r(v_nat[ci][:kn, h * DH:(h + 1) * DH]), _r(at[:kn, :]),
                                     start=(ci == 0), stop=(ci == 2))
                nc.vector.tensor_copy(out=OT[ko:ko + DH, kc, :], in_=ps_o[:, :])

            t2 = proj_T(OT, sb_wo, sb_wob)
            x1 = newact()
            for kk in range(2):
                nc.vector.tensor_tensor(out=x1[:, kk, :], in0=sb_tgt[:, kk, :], in1=t2[:, kk, :], op=ALU.add)
            x1n = layer_norm(x1, 0, 1, newact())  # norm2

            # ================= deformable attention =================
            vtab0 = single.tile([128, ST], BF16)
            vtab1 = single.tile([128, ST], BF16)
            nc.vector.memset(vtab0[:, S:], 0.0)
            nc.vector.memset(vtab1[:, S:], 0.0)
            SCH = 768
            ns_ch = (S + SCH - 1) // SCH
            if 'value' in SKIP:
                nc.vector.memset(vtab0[:, :], 0.0)
                nc.vector.memset(vtab1[:, :], 0.0)
            for si in (range(ns_ch) if 'value' not in SKIP else []):
                s0 = si * SCH
                sn = min(SCH, S - s0)
                snp = sn + (sn % 2)  # fp32r needs even moving dim
                mt_ = mpool.tile([128, 2, SCH], F32, tag="mem", name="mt_", bufs=2)
                if snp != sn:
                    for kk in range(2):
                        nc.vector.memset(mt_[:, kk, sn:snp], 0.0)
                for kk in range(2):
                    nc.sync.dma_start(out=mt_[:, kk, :sn].bitcast(F32R),
                                      in_=memT[kk * 128:(kk + 1) * 128, s0:s0 + sn].bitcast(F32R))
                for dp, vtab in ((0, vtab0), (1, vtab1)):
                    ps = vpool.tile([128, 1024], F32, tag="vwps", name="vps")
                    h0n = min(512, snp)
                    h1n = snp - h0n
                    for kk in range(2):
                        nc.tensor.matmul(ps[:, :h0n], sb_wvd[:, kk, dp * 128:(dp + 1) * 128].bitcast(F32R),
                                         mt_[:, kk, :h0n].bitcast(F32R), start=(kk == 0), stop=(kk == 1))
                    if h1n > 0:
                        for kk in range(2):
                            nc.tensor.matmul(ps[:, h0n:snp], sb_wvd[:, kk, dp * 128:(dp + 1) * 128].bitcast(F32R),
                                             mt_[:, kk, h0n:snp].bitcast(F32R), start=(kk == 0), stop=(kk == 1))
                    if si % 2 == 0:
                        nc.scalar.activation(out=vtab[:, s0:s0 + sn], in_=ps[:, :sn], func=ACT.Identity,
                                             bias=sb_vdb[:, dp, :])
                    else:
                        nc.vector.tensor_scalar(out=vtab[:, s0:s0 + sn], in0=ps[:, :sn],
                                                scalar1=sb_vdb[:, dp, :], scalar2=None, op0=ALU.add)

            q2 = newact()
            for kk in range(2):
                nc.vector.tensor_tensor(out=q2[:, kk, :], in0=x1n[:, kk, :], in1=sb_pos[:, kk, :], op=ALU.add)

            zt = single.tile([128, 320], F32, name="zt")
            nc.vector.memset(zt[:, :], 0.0)
            zp = zt[:, :]
            nc.sync.dma_start(out=bass.AP(tensor=wdram, offset=0, ap=[[1280, 128], [1, 1280]]),
                              in_=bass.AP(tensor=zp.tensor, offset=zp.offset,
                                          ap=[list(zp.ap[0]), [0, 4], [1, 320]]))
            idxs = single.tile([128, 608], U16)
            ji = [single.tile([128, 304], U16, name=f"ji{m}") for m in range(2)]
            for m in range(2):
                nc.vector.memset(ji[m][:, 300:304], 0)
            if 'samp' in SKIP:
                for m in range(2):
                    nc.vector.memset(ji[m][:, :], 0)
            for ti, (q0, qn) in (list(enumerate(QT)) if 'samp' not in SKIP else []):
                rt = tmp.tile([128, 8], F32, tag="refs")
                nc.sync.dma_start(out=rt[:qn, :], in_=refs[q0:q0 + qn, :])
                ps_off = ppool.tile([128, D], F32, tag="ps")
                for kk in range(2):
                    nc.tensor.matmul(ps_off[:qn, :], _r(q2[:, kk, q0:q0 + qn]), _r(sb_wso[:, kk, :]),
                                     start=(kk == 0), stop=(kk == 1))
                off = tmp.tile([128, D], F32, tag="off", name="off", bufs=1)
                nc.vector.tensor_tensor(out=off[:qn, :], in0=ps_off[:qn, :], in1=SOBr[:qn, :], op=ALU.add)
                ps_aw = ppool.tile([128, 128], F32, tag="pst", name="ps_aw", bufs=1)
                for kk in range(2):
                    nc.tensor.matmul(ps_aw[:qn, :], _r(q2[:, kk, q0:q0 + qn]), _r(sb_waw[:, kk, :]),
                                     start=(kk == 0), stop=(kk == 1))
                awl = tmp.tile([128, 128], F32, tag="aw")
                nc.vector.tensor_tensor(out=awl[:qn, :], in0=ps_aw[:qn, :], in1=AWBr[:qn, :], op=ALU.add)
                awv = awl[:qn, :].rearrange("p (h g) -> p h g", g=16)
                mx = tmp.tile([128, NH], F32, tag="awrow", bufs=3)
                nc.vector.tensor_reduce(out=mx[:qn, :], in_=awv, op=ALU.max, axis=AX.X)
                mxa = mx[:qn, :]
                nc.vector.tensor_tensor(out=awv, in0=awv,
                                        in1=bass.AP(tensor=mxa.tensor, offset=mxa.offset,
                                                    ap=[list(mxa.ap[0]), list(mxa.ap[1]), [0, 16]]),
                                        op=ALU.subtract)
                nc.scalar.activation(out=awl[:qn, :], in_=awl[:qn, :], func=ACT.Exp)
                sm = tmp.tile([128, NH], F32, tag="awrow", bufs=3)
                nc.vector.tensor_reduce(out=sm[:qn, :], in_=awl[:qn, :].rearrange("p (h g) -> p h g", g=16),
                                        op=ALU.add, axis=AX.X)
                rc = tmp.tile([128, NH], F32, tag="awrow", bufs=3)
                nc.vector.reciprocal(out=rc[:qn, :], in_=sm[:qn, :])
                rca = rc[:qn, :]
                aw = tmp.tile([128, 128], F32, tag="aw")
                nc.vector.tensor_tensor(out=aw[:qn, :].rearrange("p (h g) -> p h g", g=16),
                                        in0=awl[:qn, :].rearrange("p (h g) -> p h g", g=16),
                                        in1=bass.AP(tensor=rca.tensor, offset=rca.offset,
                                                    ap=[list(rca.ap[0]), list(rca.ap[1]), [0, 16]]),
                                        op=ALU.mult)

                J = tmp.tile([128, D], F32, tag="J", name="J", bufs=1)
                Wt = tmp.tile([128, 512], F32, tag="Wt", name="Wt", bufs=1)
                wxy = []
                for xy in range(2):
                    SC = WLr if xy == 0 else HLr
                    CM2 = WM2r if xy == 0 else HM2r
                    ra = rt[:qn, :]
                    refb = bass.AP(tensor=ra.tensor, offset=ra.offset + xy,
                                   ap=[list(ra.ap[0]), [0, NH], [2, NL], [0, NP]])
                    p_ = tmp.tile([128, 128], F32, tag=f"p{xy}", name="p_", bufs=2)
                    nc.vector.tensor_tensor(out=p_[:qn, :].rearrange("p (h l m) -> p h l m", h=NH, l=NL),
                                            in0=refb,
                                            in1=SC[:qn, :].rearrange("p (h l m) -> p h l m", h=NH, l=NL),
                                            op=ALU.mult)
                    offa = off[:qn, :]
                    offv = bass.AP(tensor=offa.tensor, offset=offa.offset + xy, ap=[list(offa.ap[0]), [2, 128]])
                    nc.vector.tensor_tensor(out=p_[:qn, :], in0=p_[:qn, :], in1=offv, op=ALU.add)
                    # shift +64 so trunc == floor; consts pre-shifted on host
                    nc.vector.tensor_scalar(out=p_[:qn, :], in0=p_[:qn, :], scalar1=63.5, scalar2=None, op0=ALU.add)
                    xi = tmp.tile([128, 128], mybir.dt.int32, tag="scr", name="xi", bufs=2)
                    nc.vector.tensor_copy(out=xi[:qn, :], in_=p_[:qn, :])
                    x0 = tmp.tile([128, 128], F32, tag=f"x{xy}", name="x0", bufs=1)
                    nc.vector.tensor_copy(out=x0[:qn, :], in_=xi[:qn, :])
                    # cast may trunc or round-to-nearest; fix up to floor either way
                    gt_ = tmp.tile([128, 128], F32, tag="scr", name="gt_", bufs=2)
                    nc.vector.tensor_tensor(out=gt_[:qn, :], in0=x0[:qn, :], in1=p_[:qn, :], op=ALU.is_gt)
                    nc.vector.tensor_tensor(out=x0[:qn, :], in0=x0[:qn, :], in1=gt_[:qn, :], op=ALU.subtract)
                    nc.vector.tensor_scalar(out=x0[:qn, :], in0=x0[:qn, :], scalar1=64.0, scalar2=None, op0=ALU.max)
                    nc.vector.tensor_tensor(out=x0[:qn, :], in0=x0[:qn, :], in1=CM2[:qn, :], op=ALU.min)
                    w0 = tmp.tile([128, 128], F32, tag=f"w0{xy}", name="w0", bufs=1)
                    w1_ = tmp.tile([128, 128], F32, tag=f"w1{xy}", name="w1_", bufs=1)
                    dt_ = tmp.tile([128, 128], F32, tag="scr", name="dt_", bufs=2)
                    nc.vector.tensor_tensor(out=dt_[:qn, :], in0=p_[:qn, :], in1=x0[:qn, :], op=ALU.subtract)
                    ab0 = tmp.tile([128, 128], F32, tag="scr", name="ab0", bufs=2)
                    nc.scalar.activation(out=ab0[:qn, :], in_=dt_[:qn, :], func=ACT.Abs)
                    nc.vector.tensor_scalar(out=ab0[:qn, :], in0=ab0[:qn, :], scalar1=-1.0, scalar2=1.0,
                                            op0=ALU.mult, op1=ALU.add)
                    nc.vector.tensor_scalar(out=w0[:qn, :], in0=ab0[:qn, :], scalar1=0.0, scalar2=None, op0=ALU.max)
                    nc.vector.tensor_scalar(out=dt_[:qn, :], in0=dt_[:qn, :], scalar1=-1.0, scalar2=None, op0=ALU.add)
                    nc.scalar.activation(out=ab0[:qn, :], in_=dt_[:qn, :], func=ACT.Abs)
                    nc.vector.tensor_scalar(out=ab0[:qn, :], in0=ab0[:qn, :], scalar1=-1.0, scalar2=1.0,
                                            op0=ALU.mult, op1=ALU.add)
                    nc.vector.tensor_scalar(out=w1_[:qn, :], in0=ab0[:qn, :], scalar1=0.0, scalar2=None, op0=ALU.max)
                    wxy.append((x0, w0, w1_))
                (xx0, wx0, wx1) = wxy[0]
                (yy0, wy0, wy1) = wxy[1]
                jb = tmp.tile([128, 128], F32, tag="jb", name="jb", bufs=1)
                nc.vector.tensor_tensor(out=jb[:qn, :], in0=yy0[:qn, :], in1=WLr[:qn, :], op=ALU.mult)
                nc.vector.tensor_tensor(out=jb[:qn, :], in0=jb[:qn, :], in1=xx0[:qn, :], op=ALU.add)
                nc.vector.tensor_tensor(out=jb[:qn, :], in0=jb[:qn, :], in1=LSIr[:qn, :], op=ALU.add)
                Jv = J[:qn, :].rearrange("p (f c) -> p f c", c=2)
                nc.vector.tensor_copy(out=Jv[:, :, 0], in_=jb[:qn, :])
                nc.vector.tensor_tensor(out=Jv[:, :, 1], in0=jb[:qn, :], in1=WLr[:qn, :], op=ALU.add)
                t_c = []
                for c, wyc in ((0, wy0), (1, wy1)):
                    tc_ = tmp.tile([128, 128], F32, tag=f"tc{c}", name="tc_", bufs=1)
                    nc.vector.tensor_tensor(out=tc_[:qn, :], in0=aw[:qn, :], in1=wyc[:qn, :], op=ALU.mult)
                    t_c.append(tc_)
                Wv4 = Wt[:qn, :].rearrange("p (f c s) -> p f c s", c=2, s=2)
                for c in range(2):
                    for sp, wxv in ((0, wx0), (1, wx1)):
                        nc.vector.tensor_tensor(out=Wv4[:, :, c, sp], in0=t_c[c][:qn, :], in1=wxv[:qn, :],
                                                op=ALU.mult)
                wa = Wt[:qn, :]
                nc.sync.dma_start(
                    out=bass.AP(tensor=wdram, offset=q0 * 32,
                                ap=[[32, qn], [WQPAD * 64, NH], [10240, 2], [4, 8], [2, 2], [1, 2]]),
                    in_=bass.AP(tensor=wa.tensor, offset=wa.offset,
                                ap=[list(wa.ap[0]), [64, NH], [32, 2], [4, 8], [2, 2], [1, 2]]))
                for m in range(2):
                    pst = ppool.tile([128, 128], F32, tag="pst", name="pst", bufs=1)
                    nc.tensor.transpose(pst[:, :qn], J[:qn, m * 128:(m + 1) * 128], sb_ident[:qn, :qn])
                    nc.vector.tensor_copy(out=ji[m][:, q0:q0 + qn], in_=pst[:, :qn])

            for m in range(2):
                jm = ji[m][:, :]
                nc.sync.dma_start(
                    out=bass.AP(tensor=jdram, offset=m * 38912,
                                ap=[[9728, 4], [4864, 2], [304, 16], [1, 304]]),
                    in_=bass.AP(tensor=jm.tensor, offset=jm.offset,
                                ap=[list(jm.ap[0]), [1, 304]]))
            iap = idxs[:, :]
            for hi in range(2):
                nc.sync.dma_start(
                    out=bass.AP(tensor=iap.tensor, offset=iap.offset + hi * 304,
                                ap=[list(iap.ap[0]), [1, 304]]),
                    in_=bass.AP(tensor=jdram, offset=hi * 4864,
                                ap=[[38912, 2], [9728, 4], [304, 16], [1, 304]]))

            wsb = single.tile([NH * NSEL, SUBW], F32)
            nc.sync.dma_start(out=wsb[:, :],
                              in_=bass.AP(tensor=wdram, offset=0, ap=[[WQPAD * 64, NH], [SUBW, NSEL], [1, SUBW]]))

            ODh = [[actp.tile([128, NQ], F32, name=f"OD{i}_{h}") for h in range(2)] for i in range(2)]
            HIB = [(0, 18752), (18750, 1198)]  # per level-pair table slice (elems)
            for hi in (range(2) if 'gather' not in SKIP else []):
                b0, blen = HIB[hi]
                for gi, (g0, gn) in enumerate(GCH):
                    gts = []
                    for dp, vtab in ((0, vtab0), (1, vtab1)):
                        gt = gpool.tile([128, 512, 2], BF16, tag="gather", name="gt", bufs=2)
                        nc.gpsimd.indirect_copy(
                            out=gt[:, :gn * 16, :],
                            data=vtab[:, b0:b0 + blen].rearrange("p (a b) -> p a b", b=2),
                            idxs=idxs[:, hi * 304 + g0:hi * 304 + g0 + gn],
                            i_know_ap_gather_is_preferred=True)
                        gts.append(gt)
                    nch = (gn + 31) // 32
                    for c in range(nch):
                        qq0 = g0 + c * 32
                        qqn = min(32, g0 + gn - qq0, NQ - qq0)
                        if qqn <= 0:
                            continue
                        psw = vpool.tile([128, 1024], F32, tag="vwps", name="psw")
                        for half in range(2):
                            hn = min(16, qqn - half * 16)
                            if hn <= 0:
                                continue
                            el0 = hi * 10240 + (qq0 + half * 16) * 32
                            sub = el0 // SUBW
                            eoff = el0 % SUBW
                            nc.tensor.matmul(psw[:, half * 512:half * 512 + hn * 32],
                                             _r(sb_sel[:, sub * 128:(sub + 1) * 128]),
                                             _r(wsb[:, eoff:eoff + hn * 32]), start=True, stop=True)
                        for dp in range(2):
                            gfl = gts[dp][:, (qq0 - g0) * 16:(qq0 - g0) * 16 + qqn * 16, :].rearrange(
                                "p a b -> p (a b)")
                            prod = mpool.tile([128, 1024], F32, tag="prod", name="prod", bufs=1)
                            nc.vector.tensor_tensor(out=prod[:, :qqn * 32], in0=gfl, in1=psw[:, :qqn * 32],
                                                    op=ALU.mult)
                            nc.vector.tensor_reduce(out=ODh[dp][hi][:, qq0:qq0 + qqn],
                                                    in_=prod[:, :qqn * 32].rearrange("p (a b) -> p a b", b=32),
                                                    op=ALU.add, axis=AX.X)
            if 'gather' in SKIP:
                for dp in range(2):
                    for hh in range(2):
                        nc.vector.memset(ODh[dp][hh][:, :], 0.0)
            t2d = newact()
            for mt in range(2):
                ps = ppool.tile([128, NQ], F32, tag="ps")
                for ci, (kk, hh) in enumerate([(0, 0), (0, 1), (1, 0), (1, 1)]):
                    nc.tensor.matmul(ps[:, :], _r(sb_wod[:, kk, mt * 128:(mt + 1) * 128]), _r(ODh[kk][hh][:, :]),
                                     start=(ci == 0), stop=(ci == 3))
                nc.vector.tensor_scalar(out=t2d[:, mt, :], in0=ps[:, :], scalar1=sb_wodb[:, mt, :],
                                        scalar2=None, op0=ALU.add)
            x2 = newact()
            for kk in range(2):
                nc.vector.tensor_tensor(out=x2[:, kk, :], in0=x1n[:, kk, :], in1=t2d[:, kk, :], op=ALU.add)
            x2n = layer_norm(x2, 2, 3, newact())  # norm1

            # ================= FFN =================
            h1 = actp.tile([128, 8, NQ], F32)
            if 'ffn' in SKIP:
                for mt in range(8):
                    nc.vector.memset(h1[:, mt, :], 0.0)
            for mt in (range(8) if 'ffn' not in SKIP else []):
                ps = ppool.tile([128, NQ], F32, tag="ps")
                for kk in range(2):
                    wt1 = mpool.tile([128, 128], F32, tag="w2s", name="wt1", bufs=3)
                    nc.sync.dma_start(out=wt1[:, :], in_=w1T[kk * 128:(kk + 1) * 128, mt * 128:(mt + 1) * 128])
                    nc.tensor.matmul(ps[:, :], _r(wt1[:, :]), _r(x2n[:, kk, :]),
                                     start=(kk == 0), stop=(kk == 1))
                nc.scalar.activation(out=h1[:, mt, :], in_=ps[:, :], func=ACT.Relu, bias=sb_b1[:, mt, :])
            t2f = newact()
            for mt in range(2):
                ps = ppool.tile([128, NQ], F32, tag="ps")
                for kk in range(8):
                    wt2 = mpool.tile([128, 128], F32, tag="w2s", name="w2s", bufs=3)
                    nc.sync.dma_start(out=wt2[:, :], in_=w2T[kk * 128:(kk + 1) * 128, mt * 128:(mt + 1) * 128])
                    nc.tensor.matmul(ps[:, :], _r(wt2[:, :]), _r(h1[:, kk, :]),
                                     start=(kk == 0), stop=(kk == 7))
                nc.vector.tensor_scalar(out=t2f[:, mt, :], in0=ps[:, :], scalar1=sb_b2[:, mt, :],
                                        scalar2=None, op0=ALU.add)
            x3 = newact()
            for kk in range(2):
                nc.vector.tensor_tensor(out=x3[:, kk, :], in0=x2n[:, kk, :], in1=t2f[:, kk, :], op=ALU.add)
            x3n = layer_norm(x3, 4, 5, newact())  # norm3
            for kk in range(2):
                nc.sync.dma_start(out=outT[kk * 128:(kk + 1) * 128, :], in_=x3n[:, kk, :])

    nc.compile()
    return nc


def _host_prep(inputs):
    f = lambda x: np.ascontiguousarray(np.asarray(x, dtype=np.float32))
    in_w = f(inputs["in_proj_w"]); in_b = f(inputs["in_proj_b"])
    qw, kw, vw = in_w[:D], in_w[D:2 * D], in_w[2 * D:]
    qb_, kb_, vb_ = in_b[:D], in_b[D:2 * D], in_b[2 * D:]
    sc = 1.0 / np.sqrt(DH)
    perm = np.array([h * DH + dp * 16 + r for dp in range(2) for h in range(NH) for r in range(16)])
    shared = {
        "wqT": (qw * sc).T, "wkT": kw.T, "wvT": vw.T,
        "qb": (qb_ * sc)[:, None], "kb": kb_[:, None],
        "woT": f(inputs["out_proj_w"]).T, "wob": f(inputs["out_proj_b"])[:, None],
        "wsoT": f(inputs["samp_off_w"]).T, "wawT": f(inputs["attn_wt_w"]).T,
        "wvdT": f(inputs["value_w"])[perm].T, "vdb": f(inputs["value_b"])[perm][:, None],
        "wodT": f(inputs["outp_w"]).T[perm], "wodb": f(inputs["outp_b"])[:, None],
        "w1T": f(inputs["lin1_w"]).T, "b1": f(inputs["lin1_b"])[:, None],
        "w2T": f(inputs["lin2_w"]).T, "b2": f(inputs["lin2_b"])[:, None],
        "ln_gb": np.stack([f(inputs["norm2_g"]), f(inputs["norm2_b"]),
                           f(inputs["norm1_g"]), f(inputs["norm1_b"]),
                           f(inputs["norm3_g"]), f(inputs["norm3_b"])], axis=1),
        "ident_in": np.eye(128, dtype=np.float32),
    }
    Wv_ = SPATIAL[:, 1].astype(np.float32); Hv_ = SPATIAL[:, 0].astype(np.float32)
    row = lambda vals: np.tile(np.repeat(vals, NP), NH)
    lsi_adj = LSI.astype(np.float32) - np.array([0, 0, 18750, 18750], np.float32) - 64.0 * Wv_ - 64.0
    shared["consts"] = np.stack([row(Wv_), row(Hv_), row(Wv_ + 62), row(Hv_ + 62),
                                 row(lsi_adj), f(inputs["attn_wt_b"])]).astype(np.float32)
    shared["consts2"] = np.stack([f(inputs["samp_off_b"]), vb_]).astype(np.float32)
    sel = np.zeros((NH * NSEL, NSEL * 128), dtype=np.float32)
    for s_ in range(NSEL):
        for p in range(128):
            sel[(p // 16) * NSEL + s_, s_ * 128 + p] = 1.0
    shared["sel_in"] = sel
    shared = {k: np.ascontiguousarray(np.asarray(v, np.float32)) for k, v in shared.items()}
    per_core = []
    for b in range(BS):
        m = dict(shared)
        m["tgtT"] = np.ascontiguousarray(f(inputs["tgt"][b]).T)
        m["posT"] = np.ascontiguousarray(f(inputs["tgt_query_pos"][b]).T)
        m["memT"] = np.ascontiguousarray(f(inputs["memory"][b]).T)
        m["refs"] = np.ascontiguousarray(f(inputs["tgt_reference_points"][b]).reshape(NQ, 8))
        per_core.append(m)
    return per_core


def kernel(**inputs) -> np.ndarray:
    if "nc" not in _cache:
        _cache["nc"] = build_bass()
    nc = _cache["nc"]
    in_maps = _host_prep(inputs)
    res = run_bass_kernel_spmd(nc, in_maps, core_ids=list(range(BS)))
    out = np.stack([np.ascontiguousarray(r["outT"].T) for r in res.results])
    return out.astype(np.float32)
